# revision 1
# baseline (speedup 1.0000x reference)
"""CosineAttention on 8 TRN2 NeuronCores.

Sharding: head-parallel attention + AllToAll shard-transpose +
token-parallel out-projection.

  core c owns head h=c for both batches:
    - stage 1 (per 512-token chunk): [q;k]T and vT via weight-stationary
      bf16 matmuls over xT; vT DMA-XBAR-transposed (ACT queue) into packed
      [j, 64] tiles -> [j, 65] vo tiles (ones col 64); k remapped to
      partitions 0-63; per-j k sum-of-squares via ones-matmul into PSUM;
      per-token q sum-of-squares via GpSimd partition-reduce.
    - per-batch norm: rq = 1/(sqrt(mean q^2)+eps) batched over 2048
      tokens; rk = 1/(sqrt(sum k^2)+8eps) per j (folds SCALE=1/8);
      qn = q * partition_broadcast(rq).
    - phase 2 (per 512-token i-chunk): simT[j,i] = kraw^T qn; exp on ACT
      with per-partition scale AP rk[j]; attn@[v|1] accumulates so row 64
      is the softmax denominator Z; oc = av[0:64] * partition_broadcast
      (1/Z) in bf16.  Stage-1 chunks of batch 1 interleave with batch-0
      attention to keep all engines busy.
    - ONE AllToAll [512, 512] bf16: shard g = [64, 512] feature tile for
      token block g -> core receives all 512 features for its own 512
      tokens.
    - token-parallel out-proj with the full Wout (16 matmuls) -> outT
      [512 features, 512 tokens] f32; host concatenates token blocks.
"""

import numpy as np
import ml_dtypes

import concourse.bass as bass
import concourse.tile as tile
from concourse import bacc
import concourse.mybir as mybir
from concourse import bass_utils

f32 = mybir.dt.float32
f32r = mybir.dt.float32r
bf16 = mybir.dt.bfloat16
f8 = mybir.dt.float8e4
AF = mybir.ActivationFunctionType
ALU = mybir.AluOpType

N_CORES = 8
HEADS = 8
D = 64            # head dim
B = 2             # batch
SEQ = 2048        # tokens per batch
DIM = 512         # model dim
NTOK = B * SEQ    # 4096
EPS = 1e-4
SCALE = D ** -0.5  # 0.125

CH = 512          # token chunk = A2A shard = out-proj block
NCH = NTOK // CH  # 8
JPC = CH // 128   # 4 j-tiles per chunk
JPB = SEQ // 128  # 16 j-tiles per batch

_BUILD_CACHE = {}


def build(num_devices=N_CORES, collective=True, cubic=False, newton=False):
    key = (num_devices, collective, cubic, newton)
    if key in _BUILD_CACHE:
        return _BUILD_CACHE[key]
    nc = bacc.Bacc("TRN2", target_bir_lowering=False, debug=False,
                   num_devices=num_devices)
    xT = nc.dram_tensor("xT", [DIM, NTOK], bf16, kind="ExternalInput").ap()
    wqk = nc.dram_tensor("wqk", [DIM, 128], bf16, kind="ExternalInput").ap()
    wv = nc.dram_tensor("wv", [DIM, D], bf16, kind="ExternalInput").ap()
    w2 = nc.dram_tensor("w2", [DIM, DIM], bf16, kind="ExternalInput").ap()
    o64 = nc.dram_tensor("o64", [D, 1], bf16, kind="ExternalInput").ap()
    outT = nc.dram_tensor("outT", [DIM, CH], f32, kind="ExternalOutput").ap()

    xTr = xT.rearrange("(t p) n -> p t n", p=128)
    w2r = w2.rearrange("(t p) n -> p t n", p=128)
    wqkr = wqk.rearrange("(t p) m -> p t m", p=128)
    wvr = wv.rearrange("(t p) m -> p t m", p=128)
    outTr = outT.rearrange("(mt p) n -> p mt n", p=128)

    with tile.TileContext(nc) as tc:
        with (
            tc.tile_pool(name="persist", bufs=1) as pp,
            tc.tile_pool(name="sb", bufs=2) as sb,
            tc.tile_pool(name="ps", bufs=1, space="PSUM") as ps,
            tc.tile_pool(name="dram", bufs=1, space="DRAM") as dram,
            nc.allow_low_precision(reason="bf16 matmul path"),
        ):
            # ---- persistent weights / constants ----
            wqk_sb = pp.tile([128, 4, 128], bf16)
            wv_sb = pp.tile([128, 4, D], bf16)
            w2_sb = pp.tile([128, 4, DIM], bf16)
            nc.sync.dma_start(wqk_sb[:], wqkr[:])
            nc.sync.dma_start(wv_sb[:], wvr[:])
            nc.sync.dma_start(w2_sb[:], w2r[:])
            o64_sb = pp.tile([D, 1], bf16)
            nc.sync.dma_start(o64_sb[:], o64[:])

            # ---- persistent activations ----
            xt_all = pp.tile([128, 4, NTOK], bf16)  # full xT resident
            for ci in range(NCH):
                pcols = slice(ci * CH, (ci + 1) * CH)
                nc.sync.dma_start(xt_all[:, :, pcols], xTr[:, :, pcols])
            qk_all = pp.tile([128, NTOK], bf16)   # [qT; kT] raw
            qn_sb = pp.tile([D, NTOK], bf16)      # normalized qT
            kraw_sb = pp.tile([D, NTOK], bf16)    # raw kT at partitions 0-63
            vo_sb = pp.tile([128, NCH * JPC, D + 1], bf16)  # [v | ones]
            rtq_sb = pp.tile([1, NTOK], f32)      # sqrt(mean q^2) per token
            rcqb_sb = pp.tile([1, NTOK], bf16)    # 1/(sqrt(mean q^2)+eps)
            rks_sb = pp.tile([128, NCH * JPC], f32)  # 1/(|k|+8eps) per j
            nc.gpsimd.memset(vo_sb[:, :, D:D + 1], 1.0)

            cc_in = dram.tile([DIM, CH], bf16, name="cc_in")
            cc_out = dram.tile([DIM, CH], bf16, name="cc_out")

            sks_sb = pp.tile([128, NCH * JPC], f32)  # sqrt(sum k^2) per j
            nb2_sb = pp.tile([128, 1], f32)          # exp bias -2 (fp8 range)
            nc.gpsimd.memset(nb2_sb[:], -2.0)

            def stage1_chunk(ci):
                cols = slice(ci * CH, (ci + 1) * CH)
                qk_ps = ps.tile([128, CH], f32, tag="a", bufs=3)
                for t in range(4):
                    nc.tensor.matmul(qk_ps[:], wqk_sb[:, t, :],
                                     xt_all[:, t, cols],
                                     start=(t == 0), stop=(t == 3))
                vt_ps = ps.tile([D, CH], f32, tag="b", bufs=3)
                for t in range(4):
                    nc.tensor.matmul(vt_ps[:], wv_sb[:, t, :],
                                     xt_all[:, t, cols],
                                     start=(t == 0), stop=(t == 3))
                nc.vector.tensor_copy(qk_all[:, cols], qk_ps[:])
                # raw kT at partitions 0-63 (DMA partition remap, ACT queue)
                nc.scalar.dma_start(kraw_sb[:, cols], qk_all[64:128, cols])
                # vT -> packed [j, d] tiles via DMA XBAR transpose (ACT queue)
                vt_sb = sb.tile([D, CH], bf16, tag="vtsb", bufs=3)
                nc.vector.tensor_copy(vt_sb[:], vt_ps[:])
                for jj in range(JPC):
                    jt = ci * JPC + jj
                    js = slice(jj * 128, (jj + 1) * 128)
                    vtr = sb.tile([128, D], bf16, tag="vtr", bufs=4)
                    nc.scalar.dma_start_transpose(vtr[:], vt_sb[:, js])
                    nc.gpsimd.tensor_copy(vo_sb[:, jt, 0:D], vtr[:])
                # k sum of squares per j (ones-matmul on remapped kraw)
                ksq = sb.tile([D, CH], bf16, tag="ksq", bufs=3)
                nc.vector.tensor_mul(ksq[:], kraw_sb[:, cols], kraw_sb[:, cols])
                stk_ps = ps.tile([128, JPC], f32, tag="k", bufs=1)
                for jj in range(JPC):
                    js = slice(jj * 128, (jj + 1) * 128)
                    nc.tensor.matmul(stk_ps[:, jj:jj + 1], ksq[:, js],
                                     o64_sb[:], start=True, stop=True)
                nc.scalar.activation(sks_sb[:, ci * JPC:(ci + 1) * JPC],
                                     stk_ps[:], AF.Sqrt)
                # q sum of squares per token (GpSimd partition reduce)
                sq_q = sb.tile([D, CH], bf16, tag="sqq", bufs=3)
                nc.vector.tensor_mul(sq_q[:], qk_all[0:D, cols],
                                     qk_all[0:D, cols])
                stq_ps = ps.tile([1, CH], f32, tag="s", bufs=1)
                nc.tensor.matmul(stq_ps[:], o64_sb[:], sq_q[:],
                                 start=True, stop=True)
                nc.scalar.activation(rtq_sb[:, cols], stq_ps[:], AF.Sqrt,
                                     scale=1.0 / D)
                req = sb.tile([1, CH], f32, tag="req")
                nc.vector.tensor_scalar_add(req[:], rtq_sb[:, cols], EPS)
                rcq = sb.tile([1, CH], f32, tag="rcq")
                nc.vector.reciprocal(rcq[:], req[:])
                nc.vector.tensor_copy(rcqb_sb[:, cols], rcq[:])

            def norm_half(h):
                # k reciprocal for batch h
                hs = slice(h * JPB, (h + 1) * JPB)
                seh = sb.tile([128, JPB], f32, tag="seh")
                nc.vector.tensor_scalar_add(seh[:], sks_sb[:, hs], 8.0 * EPS)
                nc.vector.reciprocal(rks_sb[:, hs], seh[:])
                # q normalization for batch h (recips precomputed per chunk)
                for cj in range(NCH // B):
                    cols = slice(h * SEQ + cj * CH, h * SEQ + (cj + 1) * CH)
                    rb_sb = sb.tile([D, CH], bf16, tag="rbsb")
                    nc.gpsimd.partition_broadcast(rb_sb[:], rcqb_sb[:, cols])
                    nc.vector.tensor_mul(qn_sb[:, cols], qk_all[0:D, cols],
                                         rb_sb[:])

            def phase2_chunk(b, ch):
                g = b * (NCH // B) + ch
                i0 = g * CH
                av_halves = []
                for half in range(2):
                    expT = sb.tile([128, JPB // 2, CH], bf16, tag="exp")
                    for jj in range(JPB // 2):
                        jt = half * (JPB // 2) + jj
                        j0 = b * SEQ + jt * 128
                        sim_ps = ps.tile([128, CH], f32, tag="a", bufs=3)
                        nc.tensor.matmul(sim_ps[:], kraw_sb[:, j0:j0 + 128],
                                         qn_sb[:, i0:i0 + CH],
                                         start=True, stop=True)
                        gj = b * JPB + jt
                        if False and cubic and jj % 3 == 2:
                            # cubic exp on DVE: |x| <= 0.125 so err < 2e-5
                            rk1 = sb.tile([128, 1], f32, tag="rk1", bufs=2)
                            nc.vector.tensor_copy(rk1[:],
                                                  rks_sb[:, gj:gj + 1])
                            tb = sb.tile([128, CH], bf16, tag="cbt", bufs=2)
                            nc.vector.tensor_scalar(
                                out=tb[:], in0=sim_ps[:],
                                scalar1=rk1[:], scalar2=None,
                                op0=ALU.mult)
                            ub = sb.tile([128, CH], bf16, tag="cbs", bufs=3)
                            nc.vector.tensor_scalar(
                                out=ub[:], in0=tb[:], scalar1=1.0 / 6.0,
                                scalar2=0.5, op0=ALU.mult, op1=ALU.add)
                            vb = sb.tile([128, CH], bf16, tag="cbs", bufs=3)
                            nc.vector.tensor_mul(vb[:], ub[:], tb[:])
                            wb = sb.tile([128, CH], bf16, tag="cbs", bufs=3)
                            nc.vector.scalar_tensor_tensor(
                                out=wb[:], in0=vb[:], scalar=1.0, in1=tb[:],
                                op0=ALU.add, op1=ALU.mult)
                            nc.vector.tensor_scalar_add(expT[:, jj, :],
                                                        wb[:], 1.0)
                        else:
                            nc.scalar.activation(expT[:, jj, :], sim_ps[:],
                                                 AF.Exp,
                                                 scale=rks_sb[:, gj:gj + 1])
                    av_ps = ps.tile([D + 1, CH], f32, tag="b", bufs=3)
                    av_halves.append(av_ps)
                    for jj in range(JPB // 2):
                        jt = half * (JPB // 2) + jj
                        nc.tensor.matmul(av_ps[:], vo_sb[:, b * JPB + jt, :],
                                         expT[:, jj, :],
                                         start=(jj == 0),
                                         stop=(jj == JPB // 2 - 1))
                av_sb = sb.tile([D + 1, CH], f32, tag="avsb")
                nc.vector.tensor_copy(av_sb[:], av_halves[0][:])
                nc.vector.tensor_tensor(av_sb[:], av_sb[:],
                                        av_halves[1][:], ALU.add)
                rse = sb.tile([1, CH], f32, tag="rse")
                if newton:
                    y0 = 1.0 / 2100.0
                    zc = sb.tile([1, CH], f32, tag="zc")
                    nc.vector.tensor_copy(zc[:], av_sb[D:D + 1, :])
                    y1 = sb.tile([1, CH], f32, tag="ny1")
                    nc.vector.tensor_scalar(out=y1[:], in0=zc[:],
                                            scalar1=-y0 * y0, scalar2=2.0 * y0,
                                            op0=ALU.mult, op1=ALU.add)
                    t2 = sb.tile([1, CH], f32, tag="nt2")
                    nc.vector.tensor_mul(t2[:], zc[:], y1[:])
                    u2 = sb.tile([1, CH], f32, tag="nu2")
                    nc.vector.tensor_scalar(out=u2[:], in0=t2[:],
                                            scalar1=-1.0, scalar2=2.0,
                                            op0=ALU.mult, op1=ALU.add)
                    nc.vector.tensor_mul(rse[:], u2[:], y1[:])
                else:
                    nc.vector.reciprocal(rse[:], av_sb[D:D + 1, :])
                r2_sb = sb.tile([D, CH], f32, tag="r2sb")
                nc.gpsimd.partition_broadcast(r2_sb[:], rse[:])
                oc = sb.tile([D, CH], bf16, tag="oc")
                nc.vector.tensor_mul(oc[:], av_sb[0:D, :], r2_sb[:])
                nc.sync.dma_start(cc_in[g * D:(g + 1) * D, :], oc[:])

            # ---- schedule ----
            for ci in range(4):
                stage1_chunk(ci)
            norm_half(0)
            for ch in range(4):
                stage1_chunk(4 + ch)
                phase2_chunk(0, ch)
            norm_half(1)
            for ch in range(4):
                phase2_chunk(1, ch)

            # ---- shard transpose: one AllToAll ----
            if collective:
                nc.gpsimd.collective_compute(
                    "AllToAll", ALU.bypass,
                    replica_groups=[list(range(num_devices))],
                    ins=[cc_in[:]], outs=[cc_out[:]])
            else:
                # timing-only stand-in (numerically wrong off-diagonal)
                nc.sync.dma_start(cc_out[:], cc_in[:])

            # ---- token-parallel out-projection ----
            ag = sb.tile([128, 4, CH], bf16, tag="ag")
            for t in range(4):
                nc.sync.dma_start(ag[:, t, :],
                                  cc_out[t * 128:(t + 1) * 128, :])
            fo = sb.tile([128, 4, CH], f32, tag="fo")
            for mt in range(4):
                fp_ps = ps.tile([128, CH], f32, tag="a", bufs=3)
                for t in range(4):
                    nc.tensor.matmul(fp_ps[:],
                                     w2_sb[:, t, mt * 128:(mt + 1) * 128],
                                     ag[:, t, :], start=(t == 0), stop=(t == 3))
                nc.vector.tensor_copy(fo[:, mt, :], fp_ps[:])
                nc.sync.dma_start(outTr[:, mt, :], fo[:, mt, :])
    nc.compile()
    _BUILD_CACHE[key] = nc
    return nc


def make_in_maps(x, Wq, Wkv, Wout):
    xT = np.ascontiguousarray(
        x.reshape(NTOK, DIM).T).astype(ml_dtypes.bfloat16)
    w2 = np.ascontiguousarray(Wout.T).astype(ml_dtypes.bfloat16)
    o64 = np.ones((D, 1), ml_dtypes.bfloat16)
    in_maps = []
    for c in range(N_CORES):
        rows = slice(c * D, (c + 1) * D)
        wqk = np.ascontiguousarray(
            np.concatenate([Wq[rows, :].T, Wkv[rows, :].T],
                           axis=1)).astype(ml_dtypes.bfloat16)
        wv = np.ascontiguousarray(
            Wkv[DIM + c * D:DIM + (c + 1) * D, :].T).astype(ml_dtypes.bfloat16)
        in_maps.append({
            "xT": xT, "wqk": wqk, "wv": wv, "w2": w2, "o64": o64,
        })
    return in_maps


def kernel(x, Wq, Wkv, Wout, _trace=False, _collective=True, _cubic=False, _newton=False):
    nc = build(collective=_collective, cubic=_cubic, newton=_newton)
    in_maps = make_in_maps(np.asarray(x), np.asarray(Wq), np.asarray(Wkv),
                           np.asarray(Wout))
    res = bass_utils.run_bass_kernel_spmd(
        nc, in_maps, core_ids=list(range(N_CORES)), trace=_trace)
    out = np.empty((NTOK, DIM), np.float32)
    for c in range(N_CORES):
        out[c * CH:(c + 1) * CH, :] = res.results[c]["outT"].T
    full = out.reshape(B, SEQ, DIM)
    if _trace:
        return full, res
    return full



# revision 19
# speedup vs baseline: 1.3128x; 1.3128x over previous
"""CosineAttention on 8 TRN2 NeuronCores — v3.

Sharding: head-parallel attention + split AllToAll shard-transpose +
token-parallel out-projection (one head per core, both batches).

Key structure (per core):
  stage 1 (per 512-token chunk): [q;k]T and vT via weight-stationary bf16
    matmuls over resident xT; vT XBAR-transposed (Sync queue) into packed
    [j, 64] vo tiles with a trailing ones column; all sqrt/rsqrt math runs
    as Ln -> Exp(scale) on ACT so ONE activation table serves the whole
    kernel (zero reloads).
  phase 2 (per 1024-token i-chunk): simT = kraw^T qn per j-tile into a
    2-bank PSUM pair, ONE exp instruction per j-tile ([128, 2x512], scale
    AP = 1/|k|), attn@[v|1] accumulates a single PSUM group; softmax
    denominator reciprocal via reciprocal_approx_fast (DVE) + a
    contraction-1 PE matmul broadcast (keeps the Pool queue free for the
    collectives).
  stage-1 work for later chunks is interleaved into phase 2 at j-tile
    granularity so neither ACT nor PE ever drains.
  TWO AllToAlls (one per batch, 256-token blocks): the first overlaps
    batch-1 attention; out-projection of batch 0 overlaps batch-1 tail.
"""

import collections

import numpy as np
import ml_dtypes

import concourse.bass as bass
import concourse.tile as tile
from concourse import bacc
import concourse.mybir as mybir
from concourse import bass_utils

f32 = mybir.dt.float32
bf16 = mybir.dt.bfloat16
AF = mybir.ActivationFunctionType
ALU = mybir.AluOpType

N_CORES = 8
HEADS = 8
D = 64            # head dim
B = 2             # batch
SEQ = 2048        # tokens per batch
DIM = 512         # model dim
NTOK = B * SEQ    # 4096

S1C = 512         # stage-1 token chunk
NS1 = NTOK // S1C          # 8
JPC = S1C // 128           # 4 j-tiles per stage-1 chunk
JPB = SEQ // 128           # 16 j-tiles per batch
P2C = 1024        # phase-2 i-chunk
HW = P2C // 2
BLK = 256         # AllToAll token block (8 blocks per batch)

_BUILD_CACHE = {}

# Steer the act-table chooser to the single table that holds BOTH ln and
# exp: keep the table list order (act_func_set_id indexes the real
# act_info.json) but hide exp/ln from every OTHER table so the chooser
# cannot alternate between exp_and_others / natural_log (each switch
# costs a 1.3us table reload).
_orig_get_tables = bacc.get_activation_tables


def _tables_force_nl_exp(arch):
    t = _orig_get_tables(arch)
    name = "natural_log_exp_and_others"
    if name not in t:
        return t
    AFT = mybir.ActivationFunctionType
    out = {}
    for k, funcs in t.items():
        if k != name:
            funcs = funcs - {AFT.Exp, AFT.Ln}
        out[k] = funcs
    return out


bacc.get_activation_tables = _tables_force_nl_exp


def build(num_devices=N_CORES, collective=True, dbg=False):
    key = (num_devices, collective, dbg)
    if key in _BUILD_CACHE:
        return _BUILD_CACHE[key]
    nc = bacc.Bacc("TRN2", target_bir_lowering=False, debug=False,
                   num_devices=num_devices)
    xT = nc.dram_tensor("xT", [DIM, NTOK], bf16, kind="ExternalInput").ap()
    wqk = nc.dram_tensor("wqk", [DIM, 128], bf16, kind="ExternalInput").ap()
    wv = nc.dram_tensor("wv", [DIM, D], bf16, kind="ExternalInput").ap()
    w2 = nc.dram_tensor("w2", [DIM, DIM], bf16, kind="ExternalInput").ap()
    o64 = nc.dram_tensor("o64", [D, 1], bf16, kind="ExternalInput").ap()
    # [512 features, 512 tokens]: cols 0:256 batch-0 block, 256:512 batch-1
    outT = nc.dram_tensor("outT", [DIM, DIM], f32, kind="ExternalOutput").ap()
    if dbg:
        d_qn = nc.dram_tensor("d_qn", [D, NTOK], bf16,
                              kind="ExternalOutput").ap()
        d_kraw = nc.dram_tensor("d_kraw", [D, NTOK], bf16,
                                kind="ExternalOutput").ap()
        d_vo = nc.dram_tensor("d_vo", [128, NS1 * JPC, D + 1], bf16,
                              kind="ExternalOutput").ap()
        d_rks = nc.dram_tensor("d_rks", [128, NS1 * JPC], f32,
                               kind="ExternalOutput").ap()
        d_oc = nc.dram_tensor("d_oc", [D, NTOK], bf16,
                              kind="ExternalOutput").ap()
        d_ag = nc.dram_tensor("d_ag", [128, 4, B * BLK], bf16,
                              kind="ExternalOutput").ap()

    xTr = xT.rearrange("(t p) n -> p t n", p=128)
    w2r = w2.rearrange("(t p) n -> p t n", p=128)
    wqkr = wqk.rearrange("(t p) m -> p t m", p=128)
    wvr = wv.rearrange("(t p) m -> p t m", p=128)
    outTr = outT.rearrange("(mt p) n -> p mt n", p=128)

    with tile.TileContext(nc) as tc:
        with (
            tc.tile_pool(name="persist", bufs=1) as pp,
            tc.tile_pool(name="sb", bufs=2) as sb,
            tc.tile_pool(name="ps", bufs=1, space="PSUM") as ps,
            tc.tile_pool(name="dram", bufs=1, space="DRAM") as dram,
            nc.allow_low_precision(reason="bf16 matmul path"),
        ):
            # ---- persistent weights / constants ----
            wqk_sb = pp.tile([128, 4, 128], bf16)
            wv_sb = pp.tile([128, 4, D], bf16)
            w2_sb = pp.tile([128, 4, DIM], bf16)
            o64_sb = pp.tile([D, 1], bf16)
            or_sb = pp.tile([1, D], f32)       # ones row for PE broadcast
            nc.gpsimd.memset(or_sb[:], 1.0)
            xt_all = pp.tile([128, 4, NTOK], bf16)  # full xT resident

            # startup DMAs: spread across queues; first-chunk pieces first
            nc.sync.dma_start(wqk_sb[:], wqkr[:])
            nc.sync.dma_start(wv_sb[:], wvr[:])
            nc.sync.dma_start(o64_sb[:], o64[:])
            c0 = slice(0, S1C)
            for t in range(4):
                nc.sync.dma_start(xt_all[:, t, c0], xTr[:, t, c0])
            for ci in range(1, 3):
                pc = slice(ci * S1C, (ci + 1) * S1C)
                nc.sync.dma_start(xt_all[:, :, pc], xTr[:, :, pc])
            for ci in range(3, 6):
                pc = slice(ci * S1C, (ci + 1) * S1C)
                nc.scalar.dma_start(xt_all[:, :, pc], xTr[:, :, pc])
            for ci in range(6, 8):
                pc = slice(ci * S1C, (ci + 1) * S1C)
                nc.gpsimd.dma_start(xt_all[:, :, pc], xTr[:, :, pc])
            nc.scalar.dma_start(w2_sb[:], w2r[:])

            # ---- persistent activations ----
            qk_all = pp.tile([128, NTOK], bf16)   # [qT; kT] raw
            qn_sb = pp.tile([D, NTOK], bf16)      # normalized qT
            kraw_sb = pp.tile([D, NTOK], bf16)    # raw kT at partitions 0-63
            vo_sb = pp.tile([128, NS1 * JPC, D + 1], bf16)  # [v | ones]
            nc.gpsimd.memset(vo_sb[:, :, D:D + 1], 1.0)
            lnk_sb = pp.tile([128, NS1 * JPC], f32)  # ln(sum k^2) per j
            rks_sb = pp.tile([128, NS1 * JPC], f32)  # 1/sqrt(sum k^2) per j
            rcq_sb = pp.tile([1, NTOK], bf16)        # 1/sqrt(mean q^2)

            cc_in = [dram.tile([DIM, BLK], bf16, name=f"cc_in{b}")
                     for b in range(B)]
            cc_out = [dram.tile([DIM, BLK], bf16, name=f"cc_out{b}")
                      for b in range(B)]

            def stage1_pieces(ci):
                cols = slice(ci * S1C, (ci + 1) * S1C)
                st = {}

                def p_qk_a():
                    st["qk_ps"] = ps.tile([128, S1C], f32, tag="s1", bufs=2,
                                          name="qk_ps")
                    for t in range(2):
                        nc.tensor.matmul(st["qk_ps"][:], wqk_sb[:, t, :],
                                         xt_all[:, t, cols],
                                         start=(t == 0), stop=False)

                def p_qk_b():
                    for t in range(2, 4):
                        nc.tensor.matmul(st["qk_ps"][:], wqk_sb[:, t, :],
                                         xt_all[:, t, cols],
                                         start=False, stop=(t == 3))
                    nc.vector.tensor_copy(qk_all[:, cols], st["qk_ps"][:])

                def p_vt():
                    vt_ps = ps.tile([D, S1C], f32, tag="s1", bufs=2,
                                    name="vt_ps")
                    for t in range(4):
                        nc.tensor.matmul(vt_ps[:], wv_sb[:, t, :],
                                         xt_all[:, t, cols],
                                         start=(t == 0), stop=(t == 3))
                    vt = sb.tile([D, S1C], bf16, tag="vtsb", bufs=2,
                                 name="vt")
                    st["vt"] = vt
                    nc.vector.tensor_copy(vt[:], vt_ps[:])

                def p_kside():
                    # raw kT remap to partitions 0-63 (Sync queue DMA)
                    nc.sync.dma_start(kraw_sb[:, cols], qk_all[64:128, cols])
                    ksq = sb.tile([D, S1C], bf16, tag="ksq", bufs=2,
                                  name="ksq")
                    nc.vector.tensor_mul(ksq[:], qk_all[64:128, cols],
                                         qk_all[64:128, cols])
                    stk_ps = ps.tile([128, JPC], f32, tag="s1", bufs=2,
                                     name="stk_ps")
                    for jj in range(JPC):
                        js = slice(jj * 128, (jj + 1) * 128)
                        nc.tensor.matmul(stk_ps[:, jj:jj + 1], ksq[:, js],
                                         o64_sb[:], start=True, stop=True)
                    nc.scalar.activation(
                        lnk_sb[:, ci * JPC:(ci + 1) * JPC], stk_ps[:], AF.Ln)

                def p_trans():
                    for jj in range(JPC):
                        jt = ci * JPC + jj
                        js = slice(jj * 128, (jj + 1) * 128)
                        vtr = sb.tile([128, D], bf16, tag="vtr", bufs=4,
                                      name="vtr")
                        nc.sync.dma_start_transpose(vtr[:], st["vt"][:, js])
                        nc.vector.tensor_copy(vo_sb[:, jt, 0:D], vtr[:])

                def p_qside():
                    sq_q = sb.tile([D, S1C], bf16, tag="sqq", bufs=2,
                                   name="sq_q")
                    nc.vector.tensor_mul(sq_q[:], qk_all[0:D, cols],
                                         qk_all[0:D, cols])
                    stq_ps = ps.tile([1, S1C], f32, tag="s1", bufs=2,
                                     name="stq_ps")
                    nc.tensor.matmul(stq_ps[:], o64_sb[:], sq_q[:],
                                     start=True, stop=True)
                    lnq = sb.tile([1, S1C], f32, tag="lnq", bufs=2,
                                  name="lnq")
                    nc.scalar.activation(lnq[:], stq_ps[:], AF.Ln,
                                         scale=1.0 / D)
                    nc.scalar.activation(rcq_sb[:, cols], lnq[:], AF.Exp,
                                         scale=-0.5)
                    rb = sb.tile([D, S1C], bf16, tag="rbsb", bufs=2,
                                 name="rb")
                    nc.gpsimd.partition_broadcast(rb[:], rcq_sb[:, cols])
                    nc.vector.tensor_mul(qn_sb[:, cols], qk_all[0:D, cols],
                                         rb[:])

                return [p_qk_a, p_qk_b, p_vt, p_kside, p_trans, p_qside]

            def rks_batch(h):
                hs = slice(h * JPB, (h + 1) * JPB)
                nc.scalar.activation(rks_sb[:, hs], lnk_sb[:, hs], AF.Exp,
                                     scale=-0.5)

            def phase2_chunk(b, c, fill):
                i0 = b * SEQ + c * P2C
                av_h = [ps.tile([D + 1, HW], f32, tag=f"av{h}", bufs=1,
                                name=f"av{h}")
                        for h in range(2)]

                def sim_mm(jj):
                    j0 = b * SEQ + jj * 128
                    sim_ps = ps.tile([128, 2, HW], f32, tag="sim", bufs=2,
                                     name="sim_ps")
                    for half in range(2):
                        ih = i0 + half * HW
                        nc.tensor.matmul(sim_ps[:, half, :],
                                         kraw_sb[:, j0:j0 + 128],
                                         qn_sb[:, ih:ih + HW],
                                         start=True, stop=True)
                    return sim_ps

                sim_ps = sim_mm(0)
                for jj in range(JPB):
                    gj = b * JPB + jj
                    expT = sb.tile([128, 2, HW], bf16, tag="exp", bufs=3,
                                   name="expT")
                    nc.scalar.activation(expT[:], sim_ps[:], AF.Exp,
                                         scale=rks_sb[:, gj:gj + 1])
                    if jj + 1 < JPB:
                        sim_ps = sim_mm(jj + 1)
                    for half in range(2):
                        nc.tensor.matmul(av_h[half][:],
                                         vo_sb[:, gj, :],
                                         expT[:, half, :],
                                         start=(jj == 0), stop=(jj == JPB - 1))
                    if fill:
                        piece = fill.popleft()
                        piece()
                # copy av out of PSUM (frees the banks for the next chunk)
                avc = sb.tile([D + 1, P2C], f32, tag="avc", bufs=2,
                              name="avc")
                for half in range(2):
                    nc.vector.tensor_copy(avc[:, half * HW:(half + 1) * HW],
                                          av_h[half][:])
                # Z = av row 64: hop to partition 0 by DMA, approx-recip,
                # then broadcast across partitions via a contraction-1
                # matmul (Pool queue stays free for the collectives)
                zrow = sb.tile([1, P2C], f32, tag="zrow", bufs=2,
                               name="zrow")
                nc.sync.dma_start(zrow[:], avc[D:D + 1, :])
                rse = sb.tile([1, P2C], f32, tag="rse", bufs=2, name="rse")
                nc.vector.reciprocal_approx_fast(out=rse[:], in_=zrow[:])
                oc = sb.tile([D, P2C], bf16, tag="oc", bufs=2, name="oc")
                for half in range(2):
                    hs = slice(half * HW, (half + 1) * HW)
                    r2_ps = ps.tile([D, HW], f32, tag="s1", bufs=2,
                                    name="r2_ps")
                    nc.tensor.matmul(r2_ps[:], or_sb[:], rse[:, hs],
                                     start=True, stop=True)
                    nc.vector.tensor_tensor(oc[:, hs], avc[0:D, hs],
                                            r2_ps[:], ALU.mult)
                # shard-transpose staging: 4 blocks of 256 tokens
                for s in range(4):
                    g = c * 4 + s
                    nc.sync.dma_start(cc_in[b][g * D:(g + 1) * D, :],
                                      oc[:, s * BLK:(s + 1) * BLK])
                if dbg:
                    nc.sync.dma_start(d_oc[:, i0:i0 + P2C], oc[:])

            def a2a(b):
                if collective:
                    nc.gpsimd.collective_compute(
                        "AllToAll", ALU.bypass,
                        replica_groups=[list(range(num_devices))],
                        ins=[cc_in[b][:]], outs=[cc_out[b][:]])
                else:
                    nc.sync.dma_start(cc_out[b][:], cc_in[b][:])

            def outproj_pieces(b):
                st = {}

                def p_ag():
                    ag = sb.tile([128, 4, BLK], bf16, tag="ag", bufs=2,
                                 name="ag")
                    st["ag"] = ag
                    nc.sync.dma_start(
                        ag[:], cc_out[b].rearrange("(t p) n -> p t n", p=128))
                    if dbg:
                        nc.sync.dma_start(d_ag[:, :, b * BLK:(b + 1) * BLK],
                                          ag[:])
                    st["fo"] = sb.tile([128, 4, BLK], f32, tag="fo", bufs=2,
                                       name="fo")

                def p_mt(mt):
                    def run():
                        fp_ps = ps.tile([128, S1C], f32, tag="s1", bufs=2,
                                        name="fp_ps")
                        for t in range(4):
                            nc.tensor.matmul(
                                fp_ps[:, 0:BLK],
                                w2_sb[:, t, mt * 128:(mt + 1) * 128],
                                st["ag"][:, t, :],
                                start=(t == 0), stop=(t == 3))
                        nc.vector.tensor_copy(st["fo"][:, mt, :],
                                              fp_ps[:, 0:BLK])
                    return run

                def p_out():
                    nc.sync.dma_start(outTr[:, :, b * BLK:(b + 1) * BLK],
                                      st["fo"][:])

                return [p_ag] + [p_mt(mt) for mt in range(4)] + [p_out]

            # ---- schedule ----
            for ci in range(4):
                for p in stage1_pieces(ci):
                    p()
            rks_batch(0)
            fill = collections.deque()
            fill.extend(stage1_pieces(4))
            fill.extend(stage1_pieces(5))
            phase2_chunk(0, 0, fill)
            fill.extend(stage1_pieces(6))
            fill.extend(stage1_pieces(7))
            fill.append(lambda: rks_batch(1))
            phase2_chunk(0, 1, fill)
            while fill:
                fill.popleft()()
            a2a(0)
            phase2_chunk(1, 0, fill)
            fill.extend(outproj_pieces(0))
            phase2_chunk(1, 1, fill)
            while fill:
                fill.popleft()()
            a2a(1)
            for p in outproj_pieces(1):
                p()
            if dbg:
                nc.sync.dma_start(d_qn[:], qn_sb[:])
                nc.sync.dma_start(d_kraw[:], kraw_sb[:])
                nc.sync.dma_start(d_vo[:], vo_sb[:])
                nc.sync.dma_start(d_rks[:], rks_sb[:])
    nc.compile()
    _BUILD_CACHE[key] = nc
    return nc


def make_in_maps(x, Wq, Wkv, Wout):
    xT = np.ascontiguousarray(
        x.reshape(NTOK, DIM).T).astype(ml_dtypes.bfloat16)
    w2 = np.ascontiguousarray(Wout.T).astype(ml_dtypes.bfloat16)
    o64 = np.ones((D, 1), ml_dtypes.bfloat16)
    in_maps = []
    for c in range(N_CORES):
        rows = slice(c * D, (c + 1) * D)
        wqk = np.ascontiguousarray(
            np.concatenate([Wq[rows, :].T, Wkv[rows, :].T],
                           axis=1)).astype(ml_dtypes.bfloat16)
        wv = np.ascontiguousarray(
            Wkv[DIM + c * D:DIM + (c + 1) * D, :].T).astype(ml_dtypes.bfloat16)
        in_maps.append({
            "xT": xT, "wqk": wqk, "wv": wv, "w2": w2, "o64": o64,
        })
    return in_maps


def kernel(x, Wq, Wkv, Wout, _trace=False, _collective=True, _dbg=False):
    nc = build(collective=_collective, dbg=_dbg)
    in_maps = make_in_maps(np.asarray(x), np.asarray(Wq), np.asarray(Wkv),
                           np.asarray(Wout))
    res = bass_utils.run_bass_kernel_spmd(
        nc, in_maps, core_ids=list(range(N_CORES)), trace=_trace)
    full = np.empty((B, SEQ, DIM), np.float32)
    for c in range(N_CORES):
        o = res.results[c]["outT"]  # [512 feat, 512 tok]
        full[0, c * BLK:(c + 1) * BLK, :] = o[:, 0:BLK].T
        full[1, c * BLK:(c + 1) * BLK, :] = o[:, BLK:2 * BLK].T
    if _trace or _dbg:
        return full, res
    return full
